# revision 11
# baseline (speedup 1.0000x reference)
"""Trainium2 Bass kernel: 8-head transformer encoder layer (B=8, S=1024,
D=300, Dh=512, H=8), data-parallel over batch across 8 NeuronCores.

v2 redesign vs the fp32 baseline:
  - all heavy matmuls run in bf16 (1 cyc/row on the PE vs fp32's ~2);
    weights/x^T are shipped from host already in bf16.
  - scores are produced TRANSPOSED: eT[t,s] = k_chunk^T(lhsT) @ qs(rhs), so
    exp() writes p^T directly in the [t, s] layout the AV matmul needs.
    This removes all 512 per-head 128x128 PE transposes and their PSUM->SBUF
    copies (~110us of the baseline).
  - the softmax scale c8[s] = gamma/sqrt(var_s + D*eps) is folded into q:
    qs = qT * bcast(c8row), where c8row comes from ROW-FORM analytic stats
    (sum_t e = ksum . q and sum_t e^2 = q^T G q via swapped-operand matmuls
    into [1, S] rows), batched across all 8 heads for the scalar math.
  - reciprocal_approx_fast everywhere (5x faster, 18-bit accurate).
  - attention is split into s-halves; the sh=1 half is interleaved with
    WO/LN1/FFN1 of sh=0 so the exp-bound attention phase overlaps the
    PE-bound FFN phase.

Math identities used (same as baseline): beta_a and the score mean drop out
by softmax shift invariance; the 1/sqrt(D) score scale cancels inside the
score LayerNorm leaving gamma/sqrt(var_raw + D*eps); v is ones-augmented so
the AV matmul also yields the softmax denominators r.
"""

import numpy as np
import ml_dtypes

import concourse.bass as bass
import concourse.tile as tile
from concourse import bacc, mybir
from concourse.bass_utils import run_bass_kernel_spmd
from concourse.masks import make_identity

F32 = mybir.dt.float32
F32R = mybir.dt.float32r
BF = mybir.dt.bfloat16
AF = mybir.ActivationFunctionType

B, S, D, DH, H, DHD = 8, 1024, 300, 512, 8, 64
DF = 4 * D  # 1200
EPS = 1e-8
NCORES = 8

DP, DFP = 384, 1280  # D/DF zero-padded to 128 multiples
J_CHUNKS = [(0, 128), (128, 128), (256, 128)]
N_ST = S // 128  # 8 s-tiles
N_SH = S // 512  # 2 s-halves
LCORR = float(D) / float(D - 1)

TRACE = False
_cache = {}
_last_results = None


def _build_nc(dbg=False):
    nc = bacc.Bacc("TRN2", debug=False)

    xd = nc.dram_tensor("x", [S, D], F32, kind="ExternalInput").ap()
    xtd = nc.dram_tensor("xt", [DP, S], BF, kind="ExternalInput").ap()
    wqd = nc.dram_tensor("wq", [DP, DH], BF, kind="ExternalInput").ap()
    wkd = nc.dram_tensor("wk", [DP, DH], BF, kind="ExternalInput").ap()
    wvd = nc.dram_tensor("wv", [DP, DH], BF, kind="ExternalInput").ap()
    wod = nc.dram_tensor("wo", [DH, D], BF, kind="ExternalInput").ap()
    w1d = nc.dram_tensor("w1", [DP, DFP], BF, kind="ExternalInput").ap()
    w2d = nc.dram_tensor("w2", [DFP, D], BF, kind="ExternalInput").ap()
    fb1d = nc.dram_tensor("fb1", [DFP, 1], F32, kind="ExternalInput").ap()
    fb2d = nc.dram_tensor("fb2", [D], F32, kind="ExternalInput").ap()
    gad = nc.dram_tensor("ga", [H, 1], F32, kind="ExternalInput").ap()
    lnd = nc.dram_tensor("ln", [4, 1], F32, kind="ExternalInput").ap()
    seld = nc.dram_tensor("sel", [H, 4, 128], BF, kind="ExternalInput").ap()
    outd = nc.dram_tensor("out", [S, D], F32, kind="ExternalOutput").ap()
    if dbg:
        dqt = nc.dram_tensor("dqt", [4, 128, S], BF, kind="ExternalOutput").ap()
        dkt = nc.dram_tensor("dkt", [4, 128, S], BF, kind="ExternalOutput").ap()
        dqs = nc.dram_tensor("dqs", [4, 128, S], BF, kind="ExternalOutput").ap()
        dat = nc.dram_tensor("dat", [4, 128, S], BF, kind="ExternalOutput").ap()
        ds1 = nc.dram_tensor("ds1", [H, S], F32, kind="ExternalOutput").ap()
        ds2 = nc.dram_tensor("ds2", [H, S], F32, kind="ExternalOutput").ap()
        dc8 = nc.dram_tensor("dc8", [H, S], BF, kind="ExternalOutput").ap()
        dx2 = nc.dram_tensor("dx2", [128, N_ST, DP], BF, kind="ExternalOutput").ap()
        dh1 = nc.dram_tensor("dh1", [10, 128, S], BF, kind="ExternalOutput").ap()
        dvv = nc.dram_tensor("dvv", [128, N_ST, H, DHD + 1], BF,
                             kind="ExternalOutput").ap()
        dgg = nc.dram_tensor("dgg", [128, H, DHD], BF, kind="ExternalOutput").ap()
        dpt = nc.dram_tensor("dpt", [2, 128, N_ST, 512], BF,
                             kind="ExternalOutput").ap()
        drw = nc.dram_tensor("drw", [2, 1, 512], F32, kind="ExternalOutput").ap()
        drb = nc.dram_tensor("drb", [2, DHD, 512], F32,
                             kind="ExternalOutput").ap()

    with tile.TileContext(nc) as tc:
        with (
            tc.tile_pool(name="wts", bufs=1) as wts,
            tc.tile_pool(name="work", bufs=1) as work,
            tc.tile_pool(name="sm", bufs=8) as sm,
            tc.tile_pool(name="ps", bufs=1, space="PSUM") as ps,
        ):
            # ---------------- constants ----------------
            ident_bf = wts.tile([128, 128], BF, tag="ident")
            make_identity(nc, ident_bf)

            ones_bf = wts.tile([128, 1], BF, tag="onesb")
            nc.vector.memset(ones_bf, 1.0)

            eps_a = wts.tile([128, 1], F32, tag="eps_a")  # D*EPS (score LN)
            nc.vector.memset(eps_a, D * EPS)
            eps_l = wts.tile([128, 1], F32, tag="eps_l")  # EPS (x LNs)
            nc.vector.memset(eps_l, EPS)

            # head-pair selectors for the c8 broadcast matmul (host constant:
            # engine memsets cannot target odd partition bases):
            # sel_t[k, hq, m] = 1 iff head k owns partition m of q-tile hq
            sel_t = wts.tile([H, 4, 128], BF, tag="sel")
            nc.sync.dma_start(out=sel_t, in_=seld)

            ga_sb = wts.tile([H, 1], F32, tag="ga")
            nc.sync.dma_start(out=ga_sb, in_=gad)

            def bcast_load(src_ap, shape, tag):
                t = wts.tile(shape, F32, tag=tag)
                nc.sync.dma_start(out=t, in_=src_ap.to_broadcast(shape))
                return t

            g1_bc = bcast_load(lnd[0:1, :], [128, 1], "g1")
            b1_bc = bcast_load(lnd[1:2, :], [128, 1], "b1")
            g2_bc = bcast_load(lnd[2:3, :], [128, 1], "g2")
            b2_bc = bcast_load(lnd[3:4, :], [128, 1], "b2")
            fb2_bc = wts.tile([128, D], F32, tag="fb2")
            nc.sync.dma_start(
                out=fb2_bc,
                in_=bass.AP(tensor=fb2d.tensor, offset=fb2d.offset,
                            ap=[[0, 128]] + list(fb2d.ap)),
            )
            fb1_sb = []
            for mt in range(10):
                t = wts.tile([128, 1], F32, tag=f"fb1_{mt}")
                nc.sync.dma_start(out=t, in_=fb1d[mt * 128 : (mt + 1) * 128, :])
                fb1_sb.append(t)

            # ---------------- input loads (bf16) ----------------
            def chunked_load(src, width, tag):
                tiles = []
                for jc, (j0, jn) in enumerate(J_CHUNKS):
                    t = wts.tile([128, width], BF, tag=f"{tag}{jc}")
                    nc.sync.dma_start(out=t, in_=src[j0 : j0 + jn, :])
                    tiles.append(t)
                return tiles

            xt_sb = chunked_load(xtd, S, "xt")    # x^T [300pad, 1024]
            wk_sb = chunked_load(wkd, DH, "wk")
            wq_sb = chunked_load(wqd, DH, "wq")
            wv_sb = chunked_load(wvd, DH, "wv")
            w1_sb = chunked_load(w1d, DFP, "w1")

            wo_sb = []
            for it in range(4):
                t = wts.tile([128, D], BF, tag=f"wo{it}")
                nc.sync.dma_start(out=t, in_=wod[it * 128 : (it + 1) * 128, :])
                wo_sb.append(t)
            w2_sb = []
            for mt in range(10):
                t = wts.tile([128, D], BF, tag=f"w2_{mt}")
                nc.sync.dma_start(out=t, in_=w2d[mt * 128 : (mt + 1) * 128, :])
                w2_sb.append(t)

            # x natural: [128, 8, 300] (partition = s % 128) for residuals
            x_sb = wts.tile([128, N_ST, D], F32, tag="x")
            nc.sync.dma_start(out=x_sb, in_=xd.rearrange("(n p) d -> p n d", p=128))

            # ---------------- persistent activations ----------------
            NBIG = 18
            qT = [work.tile([128, S], BF, tag="big", bufs=NBIG, name=f"qT{i}")
                  for i in range(4)]
            kT = [work.tile([128, S], BF, tag="big", bufs=NBIG, name=f"kT{i}")
                  for i in range(4)]
            qs = [work.tile([128, S], BF, tag="big", bufs=NBIG, name=f"qs{i}")
                  for i in range(4)]
            aT = [work.tile([128, S], BF, tag="big", bufs=NBIG, name=f"aT{i}")
                  for i in range(4)]
            kn_sb = work.tile([128, N_ST, DH], BF, tag="kn", name="kn")
            v_sb = work.tile([128, N_ST, H, DHD + 1], BF, tag="v", name="v")
            nc.vector.memset(v_sb[:, :, :, DHD : DHD + 1], 1.0)
            x2T = work.tile([128, 3, S], BF, tag="x2T", name="x2T")
            x2b = work.tile([128, N_ST, DP], BF, tag="x2b", name="x2b")
            nc.vector.memset(x2b[:, :, D:DP], 0.0)
            s1_all = wts.tile([H, S], F32, tag="s1")
            s2_all = wts.tile([H, S], F32, tag="s2")

            # ---------------- phase 1: k projection + G + ksum ----------------
            def proj_qk(dst, w_sb):
                for dt in range(4):
                    pp = ps.tile([128, 2, 512], F32, tag="e", bufs=2, name="pp")
                    for sh in range(N_SH):
                        for jc, (j0, jn) in enumerate(J_CHUNKS):
                            nc.tensor.matmul(
                                pp[:, sh, :],
                                lhsT=w_sb[jc][:, dt * 128 : (dt + 1) * 128],
                                rhs=xt_sb[jc][:, sh * 512 : (sh + 1) * 512],
                                start=(jc == 0),
                                stop=(jc == 2),
                                skip_group_check=True,
                            )
                    nc.vector.tensor_copy(
                        out=dst[dt], in_=pp.rearrange("p a b -> p (a b)"))

            proj_qk(kT, wk_sb)

            G_ps = ps.tile([DHD, H, DHD], F32, tag="av", bufs=2, name="G")
            nc.vector.memset(G_ps, 0.0)
            for stp in range(4):
                pp = ps.tile([128, 2, 512], F32, tag="e", bufs=2, name="knp")
                for t2 in range(2):
                    st = stp * 2 + t2
                    for jc, (j0, jn) in enumerate(J_CHUNKS):
                        nc.tensor.matmul(
                            pp[:, t2, :],
                            lhsT=xt_sb[jc][:, st * 128 : (st + 1) * 128],
                            rhs=wk_sb[jc],
                            start=(jc == 0),
                            stop=(jc == 2),
                            skip_group_check=True,
                        )
                nc.vector.tensor_copy(
                    out=kn_sb[:, stp * 2 : stp * 2 + 2, :], in_=pp)
                for t2 in range(2):
                    st = stp * 2 + t2
                    for h in range(H):
                        nc.tensor.matmul(
                            G_ps[:, h, :],
                            lhsT=kn_sb[:, st, h * DHD : (h + 1) * DHD],
                            rhs=kn_sb[:, st, h * DHD : (h + 1) * DHD],
                            start=False,
                            stop=(st == N_ST - 1),
                            skip_group_check=True,
                        )

            proj_qk(qT, wq_sb)

            ksum_bf = []
            for i in range(4):
                kf = sm.tile([128, 1], F32, tag="ksf", bufs=4, name=f"ksf{i}")
                nc.vector.reduce_sum(out=kf, in_=kT[i], axis=mybir.AxisListType.X)
                kb = wts.tile([128, 1], BF, tag=f"ksb{i}")
                nc.vector.tensor_copy(out=kb, in_=kf)
                ksum_bf.append(kb)

            G_sb = wts.tile([128, H, DHD], BF, tag="gsb")
            nc.vector.tensor_copy(out=G_sb[0:DHD], in_=G_ps)
            nc.sync.dma_start(out=G_sb[DHD:128], in_=G_sb[0:DHD])

            # ---------------- phase 2: row-form score stats ----------------
            def stats_pair(hq):
                y_ps = ps.tile([128, S], F32, tag="e", bufs=2, name="y")
                z_t = work.tile([128, S], BF, tag="z", bufs=2, name="z")
                for j in range(2):
                    h, hp = 2 * hq + j, j * 64
                    for sh in range(N_SH):
                        nc.tensor.matmul(
                            y_ps[hp : hp + 64, sh * 512 : (sh + 1) * 512],
                            lhsT=G_sb[hp : hp + 64, h, :],
                            rhs=qT[hq][hp : hp + 64, sh * 512 : (sh + 1) * 512],
                            start=True, stop=True, skip_group_check=True,
                        )
                    nc.vector.tensor_tensor(
                        out=z_t[hp : hp + 64, :],
                        in0=qT[hq][hp : hp + 64, :],
                        in1=y_ps[hp : hp + 64, :],
                        op=mybir.AluOpType.mult,
                    )
                    # per-head stat rows: sum_t e -> row 0, sum_t e^2 -> row 32
                    srow = ps.tile([33, S], F32, tag="e", bufs=2, name="srow")
                    for sh in range(N_SH):
                        nc.tensor.matmul(
                            srow[0:1, sh * 512 : (sh + 1) * 512],
                            lhsT=ksum_bf[hq][hp : hp + 64, :],
                            rhs=qT[hq][hp : hp + 64, sh * 512 : (sh + 1) * 512],
                            start=True, stop=True, skip_group_check=True,
                        )
                        nc.tensor.matmul(
                            srow[32:33, sh * 512 : (sh + 1) * 512],
                            lhsT=ones_bf[hp : hp + 64, :],
                            rhs=z_t[hp : hp + 64, sh * 512 : (sh + 1) * 512],
                            start=True, stop=True, skip_group_check=True,
                        )
                    srow_sb = work.tile([33, S], F32, tag="srow", bufs=2,
                                        name="srow_sb")
                    nc.vector.tensor_copy(out=srow_sb, in_=srow)
                    nc.sync.dma_start(out=s1_all[h : h + 1, :],
                                      in_=srow_sb[0:1, :])
                    nc.sync.dma_start(out=s2_all[h : h + 1, :],
                                      in_=srow_sb[32:33, :])

            for hq in range(4):
                stats_pair(hq)

            # ---------------- v projection ----------------
            def v_unit(stp):
                pp = ps.tile([128, 2, 512], F32, tag="e", bufs=2, name="vp")
                for t2 in range(2):
                    st = stp * 2 + t2
                    for jc, (j0, jn) in enumerate(J_CHUNKS):
                        nc.tensor.matmul(
                            pp[:, t2, :],
                            lhsT=xt_sb[jc][:, st * 128 : (st + 1) * 128],
                            rhs=wv_sb[jc],
                            start=(jc == 0),
                            stop=(jc == 2),
                            skip_group_check=True,
                        )
                nc.vector.tensor_copy(
                    out=v_sb[:, stp * 2 : stp * 2 + 2, :, 0:DHD],
                    in_=pp.rearrange("p a (h d) -> p a h d", h=H))

            v_unit(0)
            v_unit(1)

            # ---------------- stats math (batched over heads) ----------------
            sq = sm.tile([H, S], F32, tag="stat", bufs=2, name="sq")
            nc.vector.tensor_tensor(out=sq, in0=s1_all, in1=s1_all,
                                    op=mybir.AluOpType.mult)
            m2 = sm.tile([H, S], F32, tag="stat", bufs=2, name="m2")
            nc.vector.scalar_tensor_tensor(
                out=m2, in0=sq, scalar=-1.0 / S, in1=s2_all,
                op0=mybir.AluOpType.mult, op1=mybir.AluOpType.add)
            sd = sm.tile([H, S], F32, tag="stat", bufs=2, name="sd")
            nc.scalar.activation(out=sd, in_=m2, func=AF.Sqrt,
                                 bias=eps_a[0:H, :], scale=1.0 / (S - 1))
            rinv = sm.tile([H, S], F32, tag="stat", bufs=2, name="rinv")
            nc.vector.reciprocal_approx_fast(out=rinv, in_=sd)
            c8row = wts.tile([H, S], BF, tag="c8row")
            nc.vector.tensor_scalar_mul(c8row, rinv, ga_sb)

            v_unit(2)
            v_unit(3)

            # qs = qT * bcast(c8row)
            for hq in range(4):
                bc_ps = ps.tile([128, S], F32, tag="e", bufs=2, name="bc")
                for sh in range(N_SH):
                    nc.tensor.matmul(
                        bc_ps[:, sh * 512 : (sh + 1) * 512],
                        lhsT=sel_t[:, hq, :],
                        rhs=c8row[:, sh * 512 : (sh + 1) * 512],
                        start=True, stop=True, skip_group_check=True,
                    )
                nc.vector.tensor_tensor(out=qs[hq], in0=qT[hq], in1=bc_ps,
                                        op=mybir.AluOpType.mult)

            # ---------------- phase 3: attention ----------------
            def attn_unit(hq, sh):
                pT2 = [work.tile([128, N_ST, 512], BF, tag="pt", bufs=3,
                                 name=f"pT{j}") for j in range(2)]
                for tq2 in range(4):
                    for j in range(2):
                        hp = j * 64
                        e2 = ps.tile([128, 2, 512], F32, tag="e", bufs=2,
                                     name="e2")
                        for t2 in range(2):
                            tq = tq2 * 2 + t2
                            nc.tensor.matmul(
                                e2[:, t2, :],
                                lhsT=kT[hq][hp : hp + 64,
                                            tq * 128 : (tq + 1) * 128],
                                rhs=qs[hq][hp : hp + 64,
                                           sh * 512 : (sh + 1) * 512],
                                start=True, stop=True, skip_group_check=True,
                            )
                        nc.scalar.activation(
                            out=pT2[j][:, tq2 * 2 : tq2 * 2 + 2, :],
                            in_=e2, func=AF.Exp)
                if dbg and hq == 0 and sh == 0:
                    for j in range(2):
                        nc.sync.dma_start(out=dpt[j, :, :, :], in_=pT2[j])
                for j in range(2):
                    h, hp = 2 * hq + j, j * 64
                    av = ps.tile([DHD + 1, 512], F32, tag="av", bufs=2,
                                 name="av")
                    for tj in range(N_ST):
                        nc.tensor.matmul(
                            av,
                            lhsT=v_sb[:, tj, h, :],
                            rhs=pT2[j][:, tj, :],
                            start=(tj == 0),
                            stop=(tj == N_ST - 1),
                        )
                    # r to partition 0 first: the custom-DVE reciprocal
                    # mis-reads when in/out partition bases differ
                    r_sb = sm.tile([1, 512], F32, tag="rsb", bufs=2)
                    nc.vector.tensor_copy(out=r_sb, in_=av[DHD : DHD + 1, :])
                    rrow = sm.tile([1, 512], F32, tag="rrow", bufs=2)
                    nc.vector.reciprocal_approx_fast(out=rrow, in_=r_sb)
                    # broadcast 1/r across the 64 head dims on the idle
                    # GPSIMD engine (partition 0 -> all partitions)
                    rbc_sb = sm.tile([DHD, 512], F32, tag="rbcs", bufs=2)
                    nc.gpsimd.partition_broadcast(rbc_sb, rrow)
                    if dbg and hq == 0 and sh == 0:
                        nc.sync.dma_start(out=drw[j], in_=rrow)
                        nc.sync.dma_start(out=drb[j], in_=rbc_sb)
                    nc.vector.tensor_tensor(
                        out=aT[hq][hp : hp + 64, sh * 512 : (sh + 1) * 512],
                        in0=av[0:DHD, :], in1=rbc_sb,
                        op=mybir.AluOpType.mult,
                    )

            # ---------------- LN helper ----------------
            def ln_scalars(xr, g_bc, b_bc):
                stats = sm.tile([128, 6], F32, tag="lst", bufs=4)
                nc.vector.bn_stats(out=stats, in_=xr)
                mv = sm.tile([128, 2], F32, tag="lmv", bufs=4)
                nc.vector.bn_aggr(out=mv, in_=stats)
                sd_ = sm.tile([128, 1], F32, tag="lsd", bufs=4)
                nc.scalar.activation(out=sd_, in_=mv[:, 1:2], func=AF.Sqrt,
                                     bias=eps_l, scale=LCORR)
                rstd = sm.tile([128, 1], F32, tag="lrs", bufs=4)
                nc.vector.reciprocal_approx_fast(out=rstd, in_=sd_)
                grstd = sm.tile([128, 1], F32, tag="lgr", bufs=4)
                nc.vector.tensor_mul(grstd, rstd, g_bc)
                nb = sm.tile([128, 1], F32, tag="lnb", bufs=4)
                nc.vector.tensor_mul(nb, mv[:, 0:1], grstd)
                bias2 = sm.tile([128, 1], F32, tag="lb2", bufs=4)
                nc.vector.tensor_sub(bias2, b_bc, nb)
                return grstd, bias2

            # ---------------- WO + LN1 + x2 transpose ----------------
            def wo_unit(st):
                x1 = ps.tile([128, D], F32, tag="s", bufs=1, name="x1")
                for it in range(4):
                    nc.tensor.matmul(
                        x1,
                        lhsT=aT[it][:, st * 128 : (st + 1) * 128],
                        rhs=wo_sb[it],
                        start=(it == 0),
                        stop=(it == 3),
                    )
                xr = sm.tile([128, D], F32, tag="xr", bufs=2, name="xr1")
                nc.vector.tensor_add(xr, x1, x_sb[:, st, :])
                grstd, bias2 = ln_scalars(xr, g1_bc, b1_bc)
                nc.scalar.activation(out=x2b[:, st, 0:D], in_=xr,
                                     func=AF.Identity, bias=bias2, scale=grstd)
                xt_ps = ps.tile([128, 3, 128], BF, tag="s", bufs=1, name="xtp")
                for jc in range(3):
                    nc.tensor.transpose(
                        xt_ps[:, jc, :],
                        x2b[:, st, jc * 128 : (jc + 1) * 128],
                        ident_bf,
                    )
                nc.vector.tensor_copy(
                    out=x2T[:, :, st * 128 : (st + 1) * 128], in_=xt_ps)

            # ---------------- FFN ----------------
            def ffn1_unit(mt, sh):
                h1 = ps.tile([128, 512], F32, tag="s", bufs=1, name="h1")
                for jc in range(3):
                    nc.tensor.matmul(
                        h1,
                        lhsT=w1_sb[jc][:, mt * 128 : (mt + 1) * 128],
                        rhs=x2T[:, jc, sh * 512 : (sh + 1) * 512],
                        start=(jc == 0),
                        stop=(jc == 2),
                    )
                nc.scalar.activation(
                    out=h1T[mt][:, sh * 512 : (sh + 1) * 512],
                    in_=h1, func=AF.Relu, bias=fb1_sb[mt], scale=1.0)

            h1T = []

            def ffn2_unit(st):
                h2 = ps.tile([128, D], F32, tag="s", bufs=1, name="h2")
                for mt in range(10):
                    nc.tensor.matmul(
                        h2,
                        lhsT=h1T[mt][:, st * 128 : (st + 1) * 128],
                        rhs=w2_sb[mt],
                        start=(mt == 0),
                        stop=(mt == 9),
                    )
                xr = sm.tile([128, D], F32, tag="xr", bufs=2, name="xr2")
                nc.vector.tensor_add(xr, h2, fb2_bc)
                nc.vector.tensor_tensor(out=xr, in0=xr, in1=x2b[:, st, 0:D],
                                        op=mybir.AluOpType.add)
                grstd, bias2 = ln_scalars(xr, g2_bc, b2_bc)
                o = sm.tile([128, D], F32, tag="o", bufs=2, name="o")
                nc.scalar.activation(out=o, in_=xr, func=AF.Identity,
                                     bias=bias2, scale=grstd)
                nc.sync.dma_start(
                    out=outd[st * 128 : (st + 1) * 128, :], in_=o)

            # ---------------- schedule ----------------
            if dbg:
                for i in range(4):
                    nc.sync.dma_start(out=dqt[i], in_=qT[i])
                    nc.sync.dma_start(out=dkt[i], in_=kT[i])
                    nc.sync.dma_start(out=dqs[i], in_=qs[i])
                nc.sync.dma_start(out=ds1, in_=s1_all)
                nc.sync.dma_start(out=ds2, in_=s2_all)
                nc.sync.dma_start(out=dc8, in_=c8row)
                nc.sync.dma_start(out=dvv, in_=v_sb)
                nc.sync.dma_start(out=dgg, in_=G_sb)

            for hq in range(4):
                attn_unit(hq, 0)

            attn_unit(0, 1)
            wo_unit(0)
            attn_unit(1, 1)
            wo_unit(1)
            attn_unit(2, 1)
            wo_unit(2)
            attn_unit(3, 1)
            wo_unit(3)
            # allocated only now: the "big" ring recycles qT/kT/qs slots, so
            # h1T must not retire them before their last reads (attn sh=1)
            h1T.extend(work.tile([128, S], BF, tag="big", bufs=NBIG,
                                 name=f"h1T{i}") for i in range(10))
            wo_unit(4)
            ffn1_unit(0, 0)
            wo_unit(5)
            ffn1_unit(1, 0)
            wo_unit(6)
            ffn1_unit(2, 0)
            wo_unit(7)
            for mt in range(3, 10):
                ffn1_unit(mt, 0)
            ffn2_unit(0)
            ffn1_unit(0, 1)
            ffn2_unit(1)
            ffn1_unit(1, 1)
            ffn2_unit(2)
            ffn1_unit(2, 1)
            ffn2_unit(3)
            for mt in range(3, 10):
                ffn1_unit(mt, 1)
            for st in range(4, 8):
                ffn2_unit(st)

            if dbg:
                for i in range(4):
                    nc.sync.dma_start(out=dat[i], in_=aT[i])
                nc.sync.dma_start(out=dx2, in_=x2b)
                for i in range(10):
                    nc.sync.dma_start(out=dh1[i], in_=h1T[i])

    nc.compile()
    return nc


def _get_nc():
    if "nc" not in _cache:
        _cache["nc"] = _build_nc()
    return _cache["nc"]


def kernel(x, WQ, WK, WV, WO, W1, b1, W2, b2, gamma_a, beta_a,
           gamma1, beta1, gamma2, beta2):
    global _last_results
    f = np.float32
    bf = ml_dtypes.bfloat16
    x = np.asarray(x, f)

    def perm(W):
        # head h -> contiguous rows [h*64, (h+1)*64)
        return np.asarray(W, f).reshape(DHD, H, D).transpose(1, 0, 2).reshape(DH, D)

    def padr(a, rows, cols=None):
        out = np.zeros((rows, cols or a.shape[1]), f)
        out[: a.shape[0], : a.shape[1]] = a
        return out

    wq_t = padr(perm(WQ).T, DP).astype(bf)
    wk_t = padr(perm(WK).T, DP).astype(bf)
    wv_t = padr(perm(WV).T, DP).astype(bf)
    wo = np.ascontiguousarray(np.asarray(WO, f)).astype(bf)
    w1 = padr(np.asarray(W1, f), DP, DFP).astype(bf)
    w2 = padr(np.asarray(W2, f), DFP).astype(bf)
    fb1 = np.zeros((DFP, 1), f)
    fb1[:DF, 0] = np.asarray(b1, f)
    fb2 = np.ascontiguousarray(np.asarray(b2, f))
    ga = np.ascontiguousarray(np.asarray(gamma_a, f).reshape(H, 1))
    ln = np.array(
        [np.asarray(gamma1, f), np.asarray(beta1, f),
         np.asarray(gamma2, f), np.asarray(beta2, f)], f
    ).reshape(4, 1)

    sel_np = np.zeros((H, 4, 128), f)
    for hq in range(4):
        sel_np[2 * hq, hq, 0:64] = 1.0
        sel_np[2 * hq + 1, hq, 64:128] = 1.0
    shared = {"wq": wq_t, "wk": wk_t, "wv": wv_t, "wo": wo, "w1": w1,
              "w2": w2, "fb1": fb1, "fb2": fb2, "ga": ga, "ln": ln,
              "sel": sel_np.astype(bf)}
    in_maps = []
    for b in range(B):
        xb = np.ascontiguousarray(x[b])
        in_maps.append({"x": xb, "xt": padr(np.ascontiguousarray(xb.T), DP).astype(bf),
                        **shared})

    nc = _get_nc()
    res = run_bass_kernel_spmd(nc, in_maps, core_ids=list(range(NCORES)), trace=TRACE)
    _last_results = res
    return np.stack([res.results[b]["out"] for b in range(B)], axis=0)


# revision 12
# speedup vs baseline: 1.1186x; 1.1186x over previous
"""Trainium2 Bass kernel: 8-head transformer encoder layer (B=8, S=1024,
D=300, Dh=512, H=8), data-parallel over batch across 8 NeuronCores.

v2 redesign vs the fp32 baseline:
  - all heavy matmuls run in bf16 (1 cyc/row on the PE vs fp32's ~2);
    weights/x^T are shipped from host already in bf16.
  - scores are produced TRANSPOSED: eT[t,s] = k_chunk^T(lhsT) @ qs(rhs), so
    exp() writes p^T directly in the [t, s] layout the AV matmul needs.
    This removes all 512 per-head 128x128 PE transposes and their PSUM->SBUF
    copies (~110us of the baseline).
  - the softmax scale c8[s] = gamma/sqrt(var_s + D*eps) is folded into q:
    qs = qT * bcast(c8row), where c8row comes from ROW-FORM analytic stats
    (sum_t e = ksum . q and sum_t e^2 = q^T G q via swapped-operand matmuls
    into [1, S] rows), batched across all 8 heads for the scalar math.
  - reciprocal_approx_fast everywhere (5x faster, 18-bit accurate).
  - attention is split into s-halves; the sh=1 half is interleaved with
    WO/LN1/FFN1 of sh=0 so the exp-bound attention phase overlaps the
    PE-bound FFN phase.

Math identities used (same as baseline): beta_a and the score mean drop out
by softmax shift invariance; the 1/sqrt(D) score scale cancels inside the
score LayerNorm leaving gamma/sqrt(var_raw + D*eps); v is ones-augmented so
the AV matmul also yields the softmax denominators r.
"""

import numpy as np
import ml_dtypes

import concourse.bass as bass
import concourse.tile as tile
from concourse import bacc, mybir
from concourse.bass_utils import run_bass_kernel_spmd
from concourse.masks import make_identity

F32 = mybir.dt.float32
F32R = mybir.dt.float32r
BF = mybir.dt.bfloat16
AF = mybir.ActivationFunctionType

B, S, D, DH, H, DHD = 8, 1024, 300, 512, 8, 64
DF = 4 * D  # 1200
EPS = 1e-8
NCORES = 8

DP, DFP = 384, 1280  # D/DF zero-padded to 128 multiples
J_CHUNKS = [(0, 128), (128, 128), (256, 128)]
N_ST = S // 128  # 8 s-tiles
N_SH = S // 512  # 2 s-halves
LCORR = float(D) / float(D - 1)

TRACE = False
_cache = {}
_last_results = None


def _build_nc(dbg=False):
    nc = bacc.Bacc("TRN2", debug=False)

    xd = nc.dram_tensor("x", [S, D], F32, kind="ExternalInput").ap()
    xtd = nc.dram_tensor("xt", [DP, S], BF, kind="ExternalInput").ap()
    wqd = nc.dram_tensor("wq", [DP, DH], BF, kind="ExternalInput").ap()
    wkd = nc.dram_tensor("wk", [DP, DH], BF, kind="ExternalInput").ap()
    wvd = nc.dram_tensor("wv", [DP, DH], BF, kind="ExternalInput").ap()
    wod = nc.dram_tensor("wo", [DH, D], BF, kind="ExternalInput").ap()
    w1d = nc.dram_tensor("w1", [DP, DFP], BF, kind="ExternalInput").ap()
    w2d = nc.dram_tensor("w2", [DFP, D], BF, kind="ExternalInput").ap()
    fb1d = nc.dram_tensor("fb1", [DFP, 1], F32, kind="ExternalInput").ap()
    fb2d = nc.dram_tensor("fb2", [D], F32, kind="ExternalInput").ap()
    gad = nc.dram_tensor("ga", [H, 1], F32, kind="ExternalInput").ap()
    lnd = nc.dram_tensor("ln", [4, 1], F32, kind="ExternalInput").ap()
    seld = nc.dram_tensor("sel", [4, 2, 128], BF, kind="ExternalInput").ap()
    outd = nc.dram_tensor("out", [S, D], F32, kind="ExternalOutput").ap()
    if dbg:
        dqt = nc.dram_tensor("dqt", [4, 128, S], BF, kind="ExternalOutput").ap()
        dkt = nc.dram_tensor("dkt", [4, 128, S], BF, kind="ExternalOutput").ap()
        dqs = nc.dram_tensor("dqs", [4, 128, S], BF, kind="ExternalOutput").ap()
        dat = nc.dram_tensor("dat", [4, 128, S], BF, kind="ExternalOutput").ap()
        ds1 = nc.dram_tensor("ds1", [H, S], F32, kind="ExternalOutput").ap()
        ds2 = nc.dram_tensor("ds2", [H, S], F32, kind="ExternalOutput").ap()
        dc8 = nc.dram_tensor("dc8", [H, S], BF, kind="ExternalOutput").ap()
        dx2 = nc.dram_tensor("dx2", [128, N_ST, DP], BF, kind="ExternalOutput").ap()
        dh1 = nc.dram_tensor("dh1", [10, 128, S], BF, kind="ExternalOutput").ap()
        dvv = nc.dram_tensor("dvv", [128, N_ST, H, DHD + 1], BF,
                             kind="ExternalOutput").ap()
        dgg = nc.dram_tensor("dgg", [128, H, DHD], BF, kind="ExternalOutput").ap()
        dpt = nc.dram_tensor("dpt", [2, 128, N_ST, 512], BF,
                             kind="ExternalOutput").ap()
        drw = nc.dram_tensor("drw", [2, 1, 512], F32, kind="ExternalOutput").ap()
        drb = nc.dram_tensor("drb", [2, DHD, 512], F32,
                             kind="ExternalOutput").ap()

    with tile.TileContext(nc) as tc:
        with (
            tc.tile_pool(name="wts", bufs=1) as wts,
            tc.tile_pool(name="work", bufs=1) as work,
            tc.tile_pool(name="sm", bufs=8) as sm,
            tc.tile_pool(name="ps", bufs=1, space="PSUM") as ps,
        ):
            # ---------------- constants ----------------
            ident_bf = wts.tile([128, 128], BF, tag="ident")
            make_identity(nc, ident_bf)

            ones_bf = wts.tile([128, 1], BF, tag="onesb")
            nc.vector.memset(ones_bf, 1.0)

            eps_a = wts.tile([128, 1], F32, tag="eps_a")  # D*EPS (score LN)
            nc.vector.memset(eps_a, D * EPS)
            eps_l = wts.tile([128, 1], F32, tag="eps_l")  # EPS (x LNs)
            nc.vector.memset(eps_l, EPS)

            # ---------------- input loads (bf16) ----------------
            def chunked_load(src, width, tag):
                tiles = []
                for jc, (j0, jn) in enumerate(J_CHUNKS):
                    t = wts.tile([128, width], BF, tag=f"{tag}{jc}")
                    nc.sync.dma_start(out=t, in_=src[j0 : j0 + jn, :])
                    tiles.append(t)
                return tiles

            xt_sb = chunked_load(xtd, S, "xt")    # x^T [300pad, 1024]
            wk_sb = chunked_load(wkd, DH, "wk")
            wq_sb = chunked_load(wqd, DH, "wq")
            wv_sb = chunked_load(wvd, DH, "wv")
            w1_sb = chunked_load(w1d, DFP, "w1")

            wo_sb = []
            for it in range(4):
                t = wts.tile([128, D], BF, tag=f"wo{it}")
                nc.sync.dma_start(out=t, in_=wod[it * 128 : (it + 1) * 128, :])
                wo_sb.append(t)
            w2_sb = []
            for mt in range(10):
                t = wts.tile([128, D], BF, tag=f"w2_{mt}")
                nc.sync.dma_start(out=t, in_=w2d[mt * 128 : (mt + 1) * 128, :])
                w2_sb.append(t)

            # x natural: [128, 8, 300] (partition = s % 128) for residuals
            x_sb = wts.tile([128, N_ST, D], F32, tag="x")
            nc.sync.dma_start(out=x_sb, in_=xd.rearrange("(n p) d -> p n d", p=128))

            # small constants AFTER the big loads: their many-descriptor
            # broadcast DMAs must not delay the first projection matmuls
            # sel4[k, p, m] = 1 iff k == 2p + (m >= 64): picks the head-pair
            # rows out of a 4-head c8 batch for the bcast matmul
            sel4 = wts.tile([4, 2, 128], BF, tag="sel")
            nc.sync.dma_start(out=sel4, in_=seld)
            ga_h = []
            for b2_ in range(2):
                t = wts.tile([4, 1], F32, tag=f"ga{b2_}")
                nc.sync.dma_start(out=t, in_=gad[b2_ * 4 : b2_ * 4 + 4, :])
                ga_h.append(t)

            def bcast_load(src_ap, shape, tag):
                t = wts.tile(shape, F32, tag=tag)
                nc.sync.dma_start(out=t, in_=src_ap.to_broadcast(shape))
                return t

            g1_bc = bcast_load(lnd[0:1, :], [128, 1], "g1")
            b1_bc = bcast_load(lnd[1:2, :], [128, 1], "b1")
            g2_bc = bcast_load(lnd[2:3, :], [128, 1], "g2")
            b2_bc = bcast_load(lnd[3:4, :], [128, 1], "b2")
            fb2_bc = wts.tile([128, D], F32, tag="fb2")
            nc.sync.dma_start(
                out=fb2_bc,
                in_=bass.AP(tensor=fb2d.tensor, offset=fb2d.offset,
                            ap=[[0, 128]] + list(fb2d.ap)),
            )
            fb1_sb = []
            for mt in range(10):
                t = wts.tile([128, 1], F32, tag=f"fb1_{mt}")
                nc.sync.dma_start(out=t, in_=fb1d[mt * 128 : (mt + 1) * 128, :])
                fb1_sb.append(t)

            # ---------------- persistent activations ----------------
            NBIG = 26  # all distinct: no ring recycling hazards
            qT = [work.tile([128, S], BF, tag="big", bufs=NBIG, name=f"qT{i}")
                  for i in range(4)]
            kT = [work.tile([128, S], BF, tag="big", bufs=NBIG, name=f"kT{i}")
                  for i in range(4)]
            qs = [work.tile([128, S], BF, tag="big", bufs=NBIG, name=f"qs{i}")
                  for i in range(4)]
            aT = [work.tile([128, S], BF, tag="big", bufs=NBIG, name=f"aT{i}")
                  for i in range(4)]
            h1T = [work.tile([128, S], BF, tag="big", bufs=NBIG,
                             name=f"h1T{i}") for i in range(10)]
            kn_sb = work.tile([128, N_ST, DH], BF, tag="kn", name="kn")
            v_sb = work.tile([128, N_ST, H, DHD + 1], BF, tag="v", name="v")
            nc.vector.memset(v_sb[:, :, :, DHD : DHD + 1], 1.0)
            x2T = work.tile([128, 3, S], BF, tag="x2T", name="x2T")
            x2b = work.tile([128, N_ST, DP], BF, tag="x2b", name="x2b")
            nc.vector.memset(x2b[:, :, D:DP], 0.0)
            # score stats, split into two 4-head batches (heads 0-3 / 4-7)
            s1_t = [wts.tile([4, S], F32, tag=f"s1_{i}", name=f"s1_{i}") for i in range(2)]
            s2_t = [wts.tile([4, S], F32, tag=f"s2_{i}", name=f"s2_{i}") for i in range(2)]
            c8_t = [wts.tile([4, S], BF, tag=f"c8_{i}", name=f"c8_{i}") for i in range(2)]

            # ---------------- phase 1: k projection + G + ksum ----------------
            def proj_qk(dst, w_sb, copy_eng):
                for dt in range(4):
                    pp = ps.tile([128, 2, 512], F32, tag="e", bufs=2, name="pp")
                    for sh in range(N_SH):
                        for jc, (j0, jn) in enumerate(J_CHUNKS):
                            nc.tensor.matmul(
                                pp[:, sh, :],
                                lhsT=w_sb[jc][:, dt * 128 : (dt + 1) * 128],
                                rhs=xt_sb[jc][:, sh * 512 : (sh + 1) * 512],
                                start=(jc == 0),
                                stop=(jc == 2),
                                skip_group_check=True,
                            )
                    copy_eng(out=dst[dt], in_=pp.rearrange("p a b -> p (a b)"))

            def act_copy(out, in_):
                nc.scalar.copy(out=out, in_=in_)

            proj_qk(kT, wk_sb, act_copy)

            G_ps = ps.tile([DHD, H, DHD], F32, tag="av", bufs=2, name="G")
            nc.vector.memset(G_ps, 0.0)
            for stp in range(4):
                pp = ps.tile([128, 2, 512], F32, tag="e", bufs=2, name="knp")
                for t2 in range(2):
                    st = stp * 2 + t2
                    for jc, (j0, jn) in enumerate(J_CHUNKS):
                        nc.tensor.matmul(
                            pp[:, t2, :],
                            lhsT=xt_sb[jc][:, st * 128 : (st + 1) * 128],
                            rhs=wk_sb[jc],
                            start=(jc == 0),
                            stop=(jc == 2),
                            skip_group_check=True,
                        )
                nc.scalar.copy(
                    out=kn_sb[:, stp * 2 : stp * 2 + 2, :], in_=pp)
                for t2 in range(2):
                    st = stp * 2 + t2
                    for h in range(H):
                        nc.tensor.matmul(
                            G_ps[:, h, :],
                            lhsT=kn_sb[:, st, h * DHD : (h + 1) * DHD],
                            rhs=kn_sb[:, st, h * DHD : (h + 1) * DHD],
                            start=False,
                            stop=(st == N_ST - 1),
                            skip_group_check=True,
                        )

            proj_qk(qT, wq_sb, lambda out, in_: nc.vector.tensor_copy(
                out=out, in_=in_))

            ksum_bf = []
            for i in range(4):
                kf = sm.tile([128, 1], F32, tag="ksf", bufs=4, name=f"ksf{i}")
                nc.vector.reduce_sum(out=kf, in_=kT[i], axis=mybir.AxisListType.X)
                kb = wts.tile([128, 1], BF, tag=f"ksb{i}")
                nc.vector.tensor_copy(out=kb, in_=kf)
                ksum_bf.append(kb)

            G_sb = wts.tile([128, H, DHD], BF, tag="gsb")
            nc.vector.tensor_copy(out=G_sb[0:DHD], in_=G_ps)
            nc.sync.dma_start(out=G_sb[DHD:128], in_=G_sb[0:DHD])

            # ---------------- phase 2: row-form score stats ----------------
            def stats_pair(hq):
                y_ps = ps.tile([128, S], F32, tag="e", bufs=2, name="y")
                z_t = work.tile([128, S], BF, tag="z", bufs=2, name="z")
                for j in range(2):
                    h, hp = 2 * hq + j, j * 64
                    for sh in range(N_SH):
                        nc.tensor.matmul(
                            y_ps[hp : hp + 64, sh * 512 : (sh + 1) * 512],
                            lhsT=G_sb[hp : hp + 64, h, :],
                            rhs=qT[hq][hp : hp + 64, sh * 512 : (sh + 1) * 512],
                            start=True, stop=True, skip_group_check=True,
                        )
                    nc.vector.tensor_tensor(
                        out=z_t[hp : hp + 64, :],
                        in0=qT[hq][hp : hp + 64, :],
                        in1=y_ps[hp : hp + 64, :],
                        op=mybir.AluOpType.mult,
                    )
                    # per-head stat rows: sum_t e -> row 0, sum_t e^2 -> row 32
                    srow = ps.tile([33, S], F32, tag="e", bufs=2, name="srow")
                    for sh in range(N_SH):
                        nc.tensor.matmul(
                            srow[0:1, sh * 512 : (sh + 1) * 512],
                            lhsT=ksum_bf[hq][hp : hp + 64, :],
                            rhs=qT[hq][hp : hp + 64, sh * 512 : (sh + 1) * 512],
                            start=True, stop=True, skip_group_check=True,
                        )
                        nc.tensor.matmul(
                            srow[32:33, sh * 512 : (sh + 1) * 512],
                            lhsT=ones_bf[hp : hp + 64, :],
                            rhs=z_t[hp : hp + 64, sh * 512 : (sh + 1) * 512],
                            start=True, stop=True, skip_group_check=True,
                        )
                    srow_sb = work.tile([33, S], F32, tag="srow", bufs=2,
                                        name="srow_sb")
                    nc.scalar.copy(out=srow_sb, in_=srow)
                    hb, hr = hq // 2, 2 * (hq % 2) + j
                    nc.sync.dma_start(out=s1_t[hb][hr : hr + 1, :],
                                      in_=srow_sb[0:1, :])
                    nc.sync.dma_start(out=s2_t[hb][hr : hr + 1, :],
                                      in_=srow_sb[32:33, :])

            # ---------------- v projection ----------------
            def v_unit(stp):
                pp = ps.tile([128, 2, 512], F32, tag="e", bufs=2, name="vp")
                for t2 in range(2):
                    st = stp * 2 + t2
                    for jc, (j0, jn) in enumerate(J_CHUNKS):
                        nc.tensor.matmul(
                            pp[:, t2, :],
                            lhsT=xt_sb[jc][:, st * 128 : (st + 1) * 128],
                            rhs=wv_sb[jc],
                            start=(jc == 0),
                            stop=(jc == 2),
                            skip_group_check=True,
                        )
                nc.scalar.copy(
                    out=v_sb[:, stp * 2 : stp * 2 + 2, :, 0:DHD],
                    in_=pp.rearrange("p a (h d) -> p a h d", h=H))

            # stats math for one 4-head batch -> c8_t[hb]
            def stats_math(hb):
                sq = sm.tile([4, S], F32, tag="stat", bufs=2, name="sq")
                nc.vector.tensor_tensor(out=sq, in0=s1_t[hb], in1=s1_t[hb],
                                        op=mybir.AluOpType.mult)
                m2 = sm.tile([4, S], F32, tag="stat", bufs=2, name="m2")
                nc.vector.scalar_tensor_tensor(
                    out=m2, in0=sq, scalar=-1.0 / S, in1=s2_t[hb],
                    op0=mybir.AluOpType.mult, op1=mybir.AluOpType.add)
                sd = sm.tile([4, S], F32, tag="stat", bufs=2, name="sd")
                nc.scalar.activation(out=sd, in_=m2, func=AF.Sqrt,
                                     bias=eps_a[0:4, :], scale=1.0 / (S - 1))
                rinv = sm.tile([4, S], F32, tag="stat", bufs=2, name="rinv")
                nc.vector.reciprocal_approx_fast(out=rinv, in_=sd)
                nc.vector.tensor_scalar_mul(c8_t[hb], rinv, ga_h[hb])

            # qs = qT * bcast(c8row)
            def qs_unit(hq):
                bc_ps = ps.tile([128, S], F32, tag="e", bufs=2, name="bc")
                for sh in range(N_SH):
                    nc.tensor.matmul(
                        bc_ps[:, sh * 512 : (sh + 1) * 512],
                        lhsT=sel4[:, hq % 2, :],
                        rhs=c8_t[hq // 2][:, sh * 512 : (sh + 1) * 512],
                        start=True, stop=True, skip_group_check=True,
                    )
                nc.vector.tensor_tensor(out=qs[hq], in0=qT[hq], in1=bc_ps,
                                        op=mybir.AluOpType.mult)

            stats_pair(0)
            stats_pair(1)
            v_unit(0)
            v_unit(1)
            stats_math(0)
            stats_pair(2)
            stats_pair(3)
            qs_unit(0)
            qs_unit(1)
            v_unit(2)
            v_unit(3)
            stats_math(1)
            qs_unit(2)
            qs_unit(3)

            # ---------------- phase 3: attention ----------------
            def attn_unit(hq, sh):
                pT2 = [work.tile([128, N_ST, 512], BF, tag="pt", bufs=3,
                                 name=f"pT{j}") for j in range(2)]
                for tq2 in range(4):
                    for j in range(2):
                        hp = j * 64
                        e2 = ps.tile([128, 2, 512], F32, tag="e", bufs=2,
                                     name="e2")
                        for t2 in range(2):
                            tq = tq2 * 2 + t2
                            nc.tensor.matmul(
                                e2[:, t2, :],
                                lhsT=kT[hq][hp : hp + 64,
                                            tq * 128 : (tq + 1) * 128],
                                rhs=qs[hq][hp : hp + 64,
                                           sh * 512 : (sh + 1) * 512],
                                start=True, stop=True, skip_group_check=True,
                            )
                        nc.scalar.activation(
                            out=pT2[j][:, tq2 * 2 : tq2 * 2 + 2, :],
                            in_=e2, func=AF.Exp)
                if dbg and hq == 0 and sh == 0:
                    for j in range(2):
                        nc.sync.dma_start(out=dpt[j, :, :, :], in_=pT2[j])
                for j in range(2):
                    h, hp = 2 * hq + j, j * 64
                    av = ps.tile([DHD + 1, 512], F32, tag="av", bufs=2,
                                 name="av")
                    for tj in range(N_ST):
                        nc.tensor.matmul(
                            av,
                            lhsT=v_sb[:, tj, h, :],
                            rhs=pT2[j][:, tj, :],
                            start=(tj == 0),
                            stop=(tj == N_ST - 1),
                        )
                    # r to partition 0 first: the custom-DVE reciprocal
                    # mis-reads when in/out partition bases differ
                    r_sb = sm.tile([1, 512], F32, tag="rsb", bufs=2)
                    nc.vector.tensor_copy(out=r_sb, in_=av[DHD : DHD + 1, :])
                    rrow = sm.tile([1, 512], F32, tag="rrow", bufs=2)
                    nc.vector.reciprocal_approx_fast(out=rrow, in_=r_sb)
                    # broadcast 1/r across the 64 head dims on the idle
                    # GPSIMD engine (partition 0 -> all partitions)
                    rbc_sb = sm.tile([DHD, 512], F32, tag="rbcs", bufs=2)
                    nc.gpsimd.partition_broadcast(rbc_sb, rrow)
                    if dbg and hq == 0 and sh == 0:
                        nc.sync.dma_start(out=drw[j], in_=rrow)
                        nc.sync.dma_start(out=drb[j], in_=rbc_sb)
                    nc.vector.tensor_tensor(
                        out=aT[hq][hp : hp + 64, sh * 512 : (sh + 1) * 512],
                        in0=av[0:DHD, :], in1=rbc_sb,
                        op=mybir.AluOpType.mult,
                    )

            # ---------------- LN helper ----------------
            def ln_scalars(xr, g_bc, b_bc):
                stats = sm.tile([128, 6], F32, tag="lst", bufs=4)
                nc.vector.bn_stats(out=stats, in_=xr)
                mv = sm.tile([128, 2], F32, tag="lmv", bufs=4)
                nc.vector.bn_aggr(out=mv, in_=stats)
                sd_ = sm.tile([128, 1], F32, tag="lsd", bufs=4)
                nc.scalar.activation(out=sd_, in_=mv[:, 1:2], func=AF.Sqrt,
                                     bias=eps_l, scale=LCORR)
                rstd = sm.tile([128, 1], F32, tag="lrs", bufs=4)
                nc.vector.reciprocal_approx_fast(out=rstd, in_=sd_)
                grstd = sm.tile([128, 1], F32, tag="lgr", bufs=4)
                nc.vector.tensor_mul(grstd, rstd, g_bc)
                nb = sm.tile([128, 1], F32, tag="lnb", bufs=4)
                nc.vector.tensor_mul(nb, mv[:, 0:1], grstd)
                bias2 = sm.tile([128, 1], F32, tag="lb2", bufs=4)
                nc.vector.tensor_sub(bias2, b_bc, nb)
                return grstd, bias2

            # ---------------- WO + LN1 + x2 transpose ----------------
            def wo_unit(st):
                x1 = ps.tile([128, D], F32, tag="s", bufs=2, name="x1")
                for it in range(4):
                    nc.tensor.matmul(
                        x1,
                        lhsT=aT[it][:, st * 128 : (st + 1) * 128],
                        rhs=wo_sb[it],
                        start=(it == 0),
                        stop=(it == 3),
                    )
                xr = sm.tile([128, D], F32, tag="xr", bufs=2, name="xr1")
                nc.vector.tensor_add(xr, x1, x_sb[:, st, :])
                grstd, bias2 = ln_scalars(xr, g1_bc, b1_bc)
                nc.scalar.activation(out=x2b[:, st, 0:D], in_=xr,
                                     func=AF.Identity, bias=bias2, scale=grstd)
                xt_ps = ps.tile([128, 3, 128], BF, tag="s", bufs=2, name="xtp")
                for jc in range(3):
                    nc.tensor.transpose(
                        xt_ps[:, jc, :],
                        x2b[:, st, jc * 128 : (jc + 1) * 128],
                        ident_bf,
                    )
                nc.vector.tensor_copy(
                    out=x2T[:, :, st * 128 : (st + 1) * 128], in_=xt_ps)

            # ---------------- FFN ----------------
            def ffn1_unit(mt, sh):
                h1 = ps.tile([128, 512], F32, tag="s", bufs=2, name="h1")
                for jc in range(3):
                    nc.tensor.matmul(
                        h1,
                        lhsT=w1_sb[jc][:, mt * 128 : (mt + 1) * 128],
                        rhs=x2T[:, jc, sh * 512 : (sh + 1) * 512],
                        start=(jc == 0),
                        stop=(jc == 2),
                    )
                nc.scalar.activation(
                    out=h1T[mt][:, sh * 512 : (sh + 1) * 512],
                    in_=h1, func=AF.Relu, bias=fb1_sb[mt], scale=1.0)

            def ffn2_unit(st):
                h2 = ps.tile([128, D], F32, tag="s", bufs=2, name="h2")
                for mt in range(10):
                    nc.tensor.matmul(
                        h2,
                        lhsT=h1T[mt][:, st * 128 : (st + 1) * 128],
                        rhs=w2_sb[mt],
                        start=(mt == 0),
                        stop=(mt == 9),
                    )
                xr = sm.tile([128, D], F32, tag="xr", bufs=2, name="xr2")
                nc.vector.tensor_add(xr, h2, fb2_bc)
                nc.vector.tensor_tensor(out=xr, in0=xr, in1=x2b[:, st, 0:D],
                                        op=mybir.AluOpType.add)
                grstd, bias2 = ln_scalars(xr, g2_bc, b2_bc)
                o = sm.tile([128, D], F32, tag="o", bufs=2, name="o")
                nc.scalar.activation(out=o, in_=xr, func=AF.Identity,
                                     bias=bias2, scale=grstd)
                nc.sync.dma_start(
                    out=outd[st * 128 : (st + 1) * 128, :], in_=o)

            # ---------------- schedule ----------------
            if dbg:
                for i in range(4):
                    nc.sync.dma_start(out=dqt[i], in_=qT[i])
                    nc.sync.dma_start(out=dkt[i], in_=kT[i])
                    nc.sync.dma_start(out=dqs[i], in_=qs[i])
                for i in range(2):
                    nc.sync.dma_start(out=ds1[i * 4 : i * 4 + 4, :], in_=s1_t[i])
                    nc.sync.dma_start(out=ds2[i * 4 : i * 4 + 4, :], in_=s2_t[i])
                    nc.sync.dma_start(out=dc8[i * 4 : i * 4 + 4, :], in_=c8_t[i])
                nc.sync.dma_start(out=dvv, in_=v_sb)
                nc.sync.dma_start(out=dgg, in_=G_sb)

            for hq in range(4):
                attn_unit(hq, 0)

            attn_unit(0, 1)
            wo_unit(0)
            wo_unit(1)
            attn_unit(1, 1)
            wo_unit(2)
            wo_unit(3)
            attn_unit(2, 1)
            ffn1_unit(0, 0)
            ffn1_unit(1, 0)
            ffn1_unit(2, 0)
            attn_unit(3, 1)
            wo_unit(4)
            wo_unit(5)
            wo_unit(6)
            wo_unit(7)
            for mt in range(3, 10):
                ffn1_unit(mt, 0)
            for st in range(4):
                ffn2_unit(st)
            for mt in range(10):
                ffn1_unit(mt, 1)
            for st in range(4, 8):
                ffn2_unit(st)

            if dbg:
                for i in range(4):
                    nc.sync.dma_start(out=dat[i], in_=aT[i])
                nc.sync.dma_start(out=dx2, in_=x2b)
                for i in range(10):
                    nc.sync.dma_start(out=dh1[i], in_=h1T[i])

    nc.compile()
    return nc


def _get_nc():
    if "nc" not in _cache:
        _cache["nc"] = _build_nc()
    return _cache["nc"]


def kernel(x, WQ, WK, WV, WO, W1, b1, W2, b2, gamma_a, beta_a,
           gamma1, beta1, gamma2, beta2):
    global _last_results
    f = np.float32
    bf = ml_dtypes.bfloat16
    x = np.asarray(x, f)

    def perm(W):
        # head h -> contiguous rows [h*64, (h+1)*64)
        return np.asarray(W, f).reshape(DHD, H, D).transpose(1, 0, 2).reshape(DH, D)

    def padr(a, rows, cols=None):
        out = np.zeros((rows, cols or a.shape[1]), f)
        out[: a.shape[0], : a.shape[1]] = a
        return out

    wq_t = padr(perm(WQ).T, DP).astype(bf)
    wk_t = padr(perm(WK).T, DP).astype(bf)
    wv_t = padr(perm(WV).T, DP).astype(bf)
    wo = np.ascontiguousarray(np.asarray(WO, f)).astype(bf)
    w1 = padr(np.asarray(W1, f), DP, DFP).astype(bf)
    w2 = padr(np.asarray(W2, f), DFP).astype(bf)
    fb1 = np.zeros((DFP, 1), f)
    fb1[:DF, 0] = np.asarray(b1, f)
    fb2 = np.ascontiguousarray(np.asarray(b2, f))
    ga = np.ascontiguousarray(np.asarray(gamma_a, f).reshape(H, 1))
    ln = np.array(
        [np.asarray(gamma1, f), np.asarray(beta1, f),
         np.asarray(gamma2, f), np.asarray(beta2, f)], f
    ).reshape(4, 1)

    sel_np = np.zeros((4, 2, 128), f)
    for p_ in range(2):
        sel_np[2 * p_, p_, 0:64] = 1.0
        sel_np[2 * p_ + 1, p_, 64:128] = 1.0
    shared = {"wq": wq_t, "wk": wk_t, "wv": wv_t, "wo": wo, "w1": w1,
              "w2": w2, "fb1": fb1, "fb2": fb2, "ga": ga, "ln": ln,
              "sel": sel_np.astype(bf)}
    in_maps = []
    for b in range(B):
        xb = np.ascontiguousarray(x[b])
        in_maps.append({"x": xb, "xt": padr(np.ascontiguousarray(xb.T), DP).astype(bf),
                        **shared})

    nc = _get_nc()
    res = run_bass_kernel_spmd(nc, in_maps, core_ids=list(range(NCORES)), trace=TRACE)
    _last_results = res
    return np.stack([res.results[b]["out"] for b in range(B)], axis=0)
